# revision 1
# baseline (speedup 1.0000x reference)
"""CapsLayer2D dynamic-routing kernel for 8x TRN2 NeuronCores.

Problem (hardcoded shapes):
  inputs: [B=16, R=8, C=8, I=128, DIN=16] fp32
  W:      [K=32, I=128, DIN=16, DOUT=16] fp32
  out:    [B, R, C, K, DOUT] fp32

Math (reference does 3-round dynamic routing). Closed form (verified vs
reference to ~6e-6 rel):
  U[p,k]    = res[p,k,:,:]  (I x O per position p=(b,r,c) and k)
  s0        = mean_i U_i
  v0        = squash(s0)
  t_a = U v0        ; m_a = U^T t_a ; s1 = s0 + m_a
  v1 = squash(s1)   ; vs = v0 + v1
  t_b = U vs        ; m_b = U^T t_b ; s2 = s0 + m_b
  out = squash(s2)

Sharding: batch across 8 cores (2 batches = 128 positions per core), W
replicated. No collectives.

Per-core on-device plan:
  d padded 16->32 so each input-capsule i owns a 32-aligned partition
  block (matmul operand partition base must be a multiple of 32).
  Xt  [(i,d32) chunk c of 128 rows, p]          bf16, 32 chunks
  W_r [(i,d32) chunk c of 128 rows, (k,o)=512]  bf16, 32 chunks
  Everything after that is k-group-local (4 groups of 8 output caps):
  group g: s0_g via 32 full-depth matmuls; res_g [p, (k8,i,o)] bf16 via
  128 per-i matmuls; routing passes on DVE; out cols [g*128,(g+1)*128).
  Groups pipeline: PE produces group g+1 while DVE routes group g.
"""

import sys

import numpy as np

sys.path.insert(0, "/opt/trn_rl_repo")

import ml_dtypes  # noqa: E402

P, I, D, K, O = 128, 128, 16, 32, 16
D2 = 32  # padded d
ID, KO, KI = I * D, K * O, K * I  # 2048, 512, 4096
KC = 8  # k-group size
NG = K // KC  # 4 groups
GW = KC * O  # 128 group output width
N_CORES = 8
EPS = 1e-7

_PROGRAM = None


def _build_program():
    from contextlib import ExitStack

    import concourse.bass as bass
    import concourse.tile as tile
    from concourse import bacc, mybir

    F32 = mybir.dt.float32
    BF16 = mybir.dt.float16  # fp16: same speed class as bf16, 8x finer mantissa
    MULT = mybir.AluOpType.mult
    ADD = mybir.AluOpType.add
    X = mybir.AxisListType.X
    SQRT = mybir.ActivationFunctionType.Sqrt

    # Bacc (not raw Bass): its compile() runs generate_event_semaphores,
    # which splits multi-sem waits (TRN2 allows 1 wait per instruction).
    nc = bacc.Bacc("TRN2", target_bir_lowering=False, debug=False)

    x_d = nc.dram_tensor("x", [P, ID], F32, kind="ExternalInput").ap()
    w_d = nc.dram_tensor("w", [K, I * D * O], F32, kind="ExternalInput").ap()
    e128_d = nc.dram_tensor("e128", [128, 128], BF16, kind="ExternalInput").ap()
    out_d = nc.dram_tensor("out", [P, KO], F32, kind="ExternalOutput").ap()

    with ExitStack() as ctx:
        tc = ctx.enter_context(tile.TileContext(nc))

        pp_t = ctx.enter_context(tc.tile_pool(name="pp_t", bufs=2, space="PSUM"))
        pp_w = ctx.enter_context(tc.tile_pool(name="pp_w", bufs=2, space="PSUM"))
        pp_s = ctx.enter_context(tc.tile_pool(name="pp_s", bufs=1, space="PSUM"))
        pp_r = ctx.enter_context(tc.tile_pool(name="pp_r", bufs=3, space="PSUM"))

        rp = ctx.enter_context(tc.tile_pool(name="resp", bufs=1))
        sm0 = ctx.enter_context(tc.tile_pool(name="sm0", bufs=1))
        lhs_ctx = ExitStack()
        lhs = lhs_ctx.enter_context(tc.tile_pool(name="lhs", bufs=1))

        with tc.tile_pool(name="prep", bufs=1) as prep, tc.tile_pool(
            name="prep2", bufs=2
        ) as prep2:
            # ---- constants / x ----
            e128 = prep.tile([128, 128], BF16, tag="e128")
            nc.sync.dma_start(e128[:], e128_d)

            # cast + pad d 16->32 (zeros); chunked DMA (a 1MB DMA fans out
            # to too many HWDGE queues for one consumer's sync-wait slots)
            xb = prep.tile([P, I * D2], BF16, tag="xb")
            nc.vector.memset(xb[:], 0.0)
            xbv = xb[:].rearrange("p (i d) -> p i d", i=I)
            for q in range(4):
                xf = prep2.tile([P, ID // 4], F32, tag="xf")
                nc.sync.dma_start(xf[:], x_d[:, q * (ID // 4):(q + 1) * (ID // 4)])
                src = xf[:].rearrange("p (i d) -> p i d", i=I // 4)
                dst = xbv[:, q * (I // 4):(q + 1) * (I // 4), 0:D]
                if q % 2:
                    nc.scalar.copy(dst, src)
                else:
                    nc.vector.tensor_copy(dst, src)

            # ---- Xt: 32 chunks of [(i,d32) rows, p], chunk c at cols 128c ----
            Xt = lhs.tile([128, 32 * 128], BF16)
            for c in range(32):
                pt = pp_t.tile([128, 128], BF16, tag="pt")
                nc.tensor.transpose(pt[:], xb[:, c * 128:(c + 1) * 128], e128[:])
                if c % 2:
                    nc.scalar.copy(Xt[:, c * 128:(c + 1) * 128], pt[:])
                else:
                    nc.vector.tensor_copy(Xt[:, c * 128:(c + 1) * 128], pt[:])

            # ---- W_r: 32 chunks of [(i,d32) rows, (k,o)=512] bf16 ----
            W_r = lhs.tile([128, 32 * KO], BF16)
            # two persistent cast+pad staging buffers; pads zeroed once
            wb0 = prep.tile([K, 4 * D2 * O], BF16, tag="wb0")
            wb1 = prep.tile([K, 4 * D2 * O], BF16, tag="wb1")
            wbs = [wb0, wb1]
            nc.vector.memset(wbs[0][:], 0.0)
            nc.vector.memset(wbs[1][:], 0.0)
            for c in range(32):
                # chunk covers i in [4c, 4c+4): raw [32, 4*D*O=1024] fp32
                wf = prep2.tile([K, 4 * D * O], F32, tag="wf")
                nc.sync.dma_start(wf[:], w_d[:, c * 1024:(c + 1) * 1024])
                wb = wbs[c % 2]
                wdst = wb[:].rearrange("k (i d o) -> k i d o", i=4, d=D2)[
                    :, :, 0:D, :
                ]
                wsrc = wf[:].rearrange("k (i d o) -> k i d o", i=4, d=D)
                if c % 2:
                    nc.scalar.copy(wdst, wsrc)
                else:
                    nc.vector.tensor_copy(wdst, wsrc)
                wv = wb[:].rearrange("k (i d o) -> k i d o", i=4, d=D2)
                pw = pp_w.tile([128, KO], BF16, tag="pw")
                for o0 in range(O):
                    # in: [32, (i4,d32)=128] at fixed o -> out [128, 32]
                    nc.tensor.transpose(
                        pw[:, o0 * 32:(o0 + 1) * 32],
                        wv[:, :, :, o0],
                        e128[0:32, 0:32],
                    )
                # psum cols are (o,k); store as (k,o)
                src = (
                    pw[:]
                    .rearrange("p (o k) -> p o k", o=O, k=K)
                    .transpose([0, 2, 1])
                )
                dst = W_r[:, c * KO:(c + 1) * KO].rearrange(
                    "p (k o) -> p k o", k=K
                )
                if c % 2:
                    nc.scalar.copy(dst, src)
                else:
                    nc.vector.tensor_copy(dst, src)

        # ---- production: s0 (all k) + res (all k), then routing per group ----
        ps0 = pp_s.tile([P, KO], F32, tag="ps0")
        for c in range(32):
            nc.tensor.matmul(
                ps0[:],
                Xt[:, c * 128:(c + 1) * 128],
                W_r[:, c * KO:(c + 1) * KO],
                start=(c == 0),
                stop=(c == 31),
            )
        s0_all = sm0.tile([P, KO], F32)
        nc.scalar.mul(s0_all[:], ps0[:], 1.0 / I)

        res = rp.tile([P, K * I * O], BF16)
        resv = res[:].rearrange("p (k i o) -> p k i o", k=K, i=I, o=O)
        for i in range(I):
            c, r0 = i // 4, (i % 4) * 32
            pr = pp_r.tile([P, KO], F32, tag="pr")  # one full PSUM bank
            nc.tensor.matmul(
                pr[:],
                Xt[r0:r0 + 32, c * 128:(c + 1) * 128],
                W_r[r0:r0 + 32, c * KO:(c + 1) * KO],
                start=True,
                stop=True,
                tile_position=(r0, 0),
            )
            src_ap = pr[:].rearrange("p (k o) -> p k o", k=K)
            if i % 2:
                nc.scalar.copy(resv[:, :, i, :], src_ap)
            else:
                nc.vector.tensor_copy(resv[:, :, i, :], src_ap)

        # ---- routing (Xt/W_r freed) ----
        lhs_ctx.close()
        sm = ctx.enter_context(tc.tile_pool(name="small", bufs=1))
        eps_t = sm.tile([P, 1], F32, tag="eps")
        nc.vector.memset(eps_t[:], EPS)

        def squash(s_ap, v_ap, tag):
            ssq = sm.tile([P, GW], F32, tag="ssq")
            nc.vector.tensor_mul(ssq[:], s_ap, s_ap)
            sq = sm.tile([P, KC], F32, tag=f"sq_{tag}")
            nc.vector.tensor_reduce(
                sq[:], ssq[:].rearrange("p (k o) -> p k o", k=KC), X, ADD
            )
            a = sm.tile([P, KC], F32, tag="sqa")
            nc.scalar.activation(a[:], sq[:], SQRT, bias=eps_t[:])
            b = sm.tile([P, KC], F32, tag="sqb")
            nc.vector.scalar_tensor_tensor(b[:], sq[:], 1.0, a[:], ADD, MULT)
            r = sm.tile([P, KC], F32, tag="sqr")
            nc.vector.reciprocal(r[:], b[:])
            f = sm.tile([P, KC], F32, tag="sqf")
            nc.vector.tensor_mul(f[:], sq[:], r[:])
            nc.vector.tensor_mul(
                v_ap.rearrange("p (k o) -> p k o", k=KC),
                s_ap.rearrange("p (k o) -> p k o", k=KC),
                f[:].unsqueeze(2).broadcast_to([P, KC, O]),
            )

        for g in range(NG):
            rv = resv[:, g * KC:(g + 1) * KC]
            s0 = s0_all[:, g * GW:(g + 1) * GW]

            def uv_pass(vb_t, t_t):
                """t = U v (contract o)."""
                tmp = sm.tile([P, KC * I * O], BF16, tag="tmp")
                tmpv = tmp[:].rearrange("p (k i o) -> p k i o", k=KC, i=I, o=O)
                nc.vector.tensor_mul(
                    tmpv,
                    rv,
                    vb_t[:]
                    .rearrange("p (k o) -> p k o", k=KC)
                    .unsqueeze(2)
                    .broadcast_to([P, KC, I, O]),
                )
                nc.vector.tensor_reduce(
                    t_t[:].rearrange("p (k i) -> p k i", k=KC), tmpv, X, ADD
                )

            def ut_pass(t_t, m_t):
                """m = U^T t (contract i)."""
                tmp = sm.tile([P, KC * I * O], BF16, tag="tmp")
                tmp_kio = (
                    tmp[:]
                    .rearrange("p (k o i) -> p k o i", k=KC, o=O, i=I)
                    .transpose([0, 1, 3, 2])
                )
                nc.vector.tensor_mul(
                    tmp_kio,
                    rv,
                    t_t[:]
                    .rearrange("p (k i) -> p k i", k=KC)
                    .unsqueeze(3)
                    .broadcast_to([P, KC, I, O]),
                )
                tmp_koi = tmp[:].rearrange(
                    "p (k o i) -> p k o i", k=KC, o=O, i=I
                )
                nc.vector.tensor_reduce(
                    m_t[:].rearrange("p (k o) -> p k o", k=KC), tmp_koi, X, ADD
                )

            with nc.allow_low_precision(reason="fp16 routing intermediates"):
                v0 = sm.tile([P, GW], F32, tag="v0")
                squash(s0, v0[:], "v0")
                v0b = sm.tile([P, GW], BF16, tag="v0b")
                nc.vector.tensor_copy(v0b[:], v0[:])

                t_a = sm.tile([P, KC * I], BF16, tag="t")
                uv_pass(v0b, t_a)
                m_a = sm.tile([P, GW], BF16, tag="m")
                ut_pass(t_a, m_a)

                s1 = sm.tile([P, GW], F32, tag="s1")
                nc.vector.tensor_add(s1[:], s0, m_a[:])
                v1 = sm.tile([P, GW], F32, tag="v1")
                squash(s1[:], v1[:], "v1")
                vs = sm.tile([P, GW], F32, tag="vs")
                nc.vector.tensor_add(vs[:], v0[:], v1[:])
                vsb = sm.tile([P, GW], BF16, tag="vsb")
                nc.vector.tensor_copy(vsb[:], vs[:])

                t_b = sm.tile([P, KC * I], BF16, tag="t")
                uv_pass(vsb, t_b)
                m_b = sm.tile([P, GW], BF16, tag="m")
                ut_pass(t_b, m_b)

                s2 = sm.tile([P, GW], F32, tag="s2")
                nc.vector.tensor_add(s2[:], s0, m_b[:])
                outt = sm.tile([P, GW], F32, tag="outt")
                squash(s2[:], outt[:], "out")

            nc.sync.dma_start(out_d[:, g * GW:(g + 1) * GW], outt[:])

    nc.compile()
    return nc


def _get_program():
    global _PROGRAM
    if _PROGRAM is None:
        _PROGRAM = _build_program()
    return _PROGRAM


def kernel(**inputs):
    x = np.ascontiguousarray(np.asarray(inputs["inputs"], dtype=np.float32))
    W = np.ascontiguousarray(np.asarray(inputs["W"], dtype=np.float32))
    assert x.shape == (16, 8, 8, 128, 16) and W.shape == (32, 128, 16, 16)

    from concourse.bass_utils import run_bass_kernel_spmd

    nc = _get_program()

    xs = x.reshape(N_CORES, P, ID)  # [core, p=128, (i,d)]
    wflat = np.ascontiguousarray(W.reshape(K, I * D * O))
    e128 = np.eye(128, dtype=np.float16)
    in_maps = [
        {"x": np.ascontiguousarray(xs[c]), "w": wflat, "e128": e128}
        for c in range(N_CORES)
    ]
    r = run_bass_kernel_spmd(nc, in_maps, list(range(N_CORES)))
    outs = [r.results[c]["out"].reshape(2, 8, 8, K, O) for c in range(N_CORES)]
    return np.concatenate(outs, axis=0).astype(np.float32)

